# revision 34
# baseline (speedup 1.0000x reference)
"""Trainium2 Bass kernel for nn_Net_53644141527184.

Computation (per batch b):
  For each branch br in {x1, x3, x5}:
    picked[b, g, p] = x_br[b, idx[br, g, p], p]          (channel gather, p = 0..255)
    grid = picked.reshape(B, 128, 16, 16)
    crop[b, g, i, j] = grid[b, g, oh[g]+i, ow[g]+j]      (per-group 14x14 crop)
  feats = concat(crops, axis=1)                          -> [B, 384, 14, 14]
  out = einsum('bchw,oc->bohw', feats, W)                -> [B, 768, 14, 14]

Strategy: pure data parallel over 8 NeuronCores (16 batches each).
x is relaid out host-side (pure data movement) to [k=32, (pl 8 x b 16)=128, c=512]
so each SBUF tile has partition = (position, batch); groups are stable-sorted
host-side by crop offset v = 2*oh+ow into 32-aligned padded slot bands (pad
slots gather channel 0 and carry zero W rows, contributing nothing).

Per core pipeline:
  1. DMA x tiles [128, KB*512] f32 (2KB contiguous runs), KB=8 k-blocks each
     (first/last tiles split in half for pipeline ramp/tail).
  2. One merged gpsimd.ap_gather per tile: indices ki*512+c pick channels for
     all KB k-blocks in one scan -> pk [(pl,b), (ki, slot)] f32.
  3. PE transpose per (row-pair, 128-slot chunk) into a full 2KB PSUM bank
     [slot, (k4, pl, b)] = two grid rows.
  4. Crop fused into PSUM->SBUF copies: per offset band, a 4D strided copy
     moves both rows' 14 valid columns at the band's constant shift into
     conv-ready feats tiles [slot, (b, q')] bf16, split into 4 q-row groups
     (qr 0-6, 7-10, 11-12, 13) so conv work unlocks progressively.
  5. 1x1 conv per row group: psum[o, (b, q')] accumulated over K tiles of
     bf16 matmuls, drained to bf16 staging, DMA'd out (f32 upcast on host).
     Out-DMAs are issued on the same queue after all x DMAs so they never
     preempt the input stream.
Index arrays / W / x are preprocessed host-side into device-friendly layouts
(pure relayout + address arithmetic; x values are never touched).
"""

import numpy as np
from contextlib import ExitStack

import ml_dtypes

import concourse.bacc as bacc
import concourse.bass as bass
import concourse.tile as tile
import concourse.mybir as mybir
from concourse import bass_utils, masks

N_CORES = 8
B = 16        # batches per core
C = 512
P = 256       # grid positions (16x16)
G = 128       # groups per branch
BR = 3
OC = 768
NK = 32       # position blocks of 8 (k = 2*row + col_half)
KB = 4        # k-blocks per gather tile
NKO = NK // KB
# grid row 15 (k-blocks 30, 31) is never read by any crop (offsets in
# {0,1}, crop height 14): skip its DMA + gather entirely

# conv row groups: q-rows [LO[g], LO[g+1]) per group, sized so the groups
# gated on the last gathers (ko6/ko7) carry minimal conv work
GLO = [0, 5, 8, 11, 13, 14]
NG = len(GLO) - 1
GWQ = [14 * (GLO[g + 1] - GLO[g]) for g in range(NG)]  # [70, 42, 42, 28, 14]
GNB = [4, 8, 8, 8, 8]                                  # batches per matmul

_CACHE = {}


def _qgroup(qr):
    for g in range(NG):
        if GLO[g] <= qr < GLO[g + 1]:
            return g
    return None


def _plan(offh, offw):
    """Compute the padded slot layout and conv-tile packing."""
    v = 2 * offh.astype(int) + offw.astype(int)
    perms = [np.argsort(v[br], kind="stable") for br in range(BR)]
    plan = {"perms": perms, "S": [], "bands": [], "pieces": []}
    for br in range(BR):
        cnt = np.bincount(v[br], minlength=4)
        slot = 0
        bands = []
        for vv in range(4):
            n = int(cnt[vv])
            bands.append((vv, slot, n))
            slot += ((n + 31) // 32) * 32
        S = max(slot, 128)
        if S % 4:
            S += 4 - (S % 4)
        plan["S"].append(S)
        plan["bands"].append(bands)

    # residual chunks (slots >= 128) pack greedily into extra tiles
    resid_assign = {}
    bins = []
    for br in range(BR):
        sz = plan["S"][br] - 128
        sz = ((sz + 31) // 32) * 32
        if sz <= 0:
            continue
        placed = False
        for i in range(len(bins)):
            if bins[i] + sz <= 128:
                resid_assign[br] = (BR + i, bins[i])
                bins[i] += sz
                placed = True
                break
        if not placed:
            bins.append(sz)
            resid_assign[br] = (BR + len(bins) - 1, 0)
    n_tiles = BR + len(bins)
    plan["n_tiles"] = n_tiles

    # copy pieces: band slot sub-ranges -> (tile, tile partition offset).
    # Copies are extended over the 32-alignment pad rows (their data is a
    # harmless finite duplicate; their W rows are zero).  Engine partition
    # windows are buddy-aligned: from base b != 0 an access must not cross
    # the b + (b & -b) boundary; base 0 is unrestricted.
    used_rows = [0] * n_tiles
    for br in range(BR):
        for (vv, slot_lo, n) in plan["bands"][br]:
            lo = slot_lo
            remaining = ((n + 31) // 32) * 32
            while remaining > 0:
                chunk = lo // 128
                in_chunk = lo % 128
                take = min(remaining, 128 - in_chunk)
                if chunk == 0:
                    tid, tofs = br, in_chunk
                else:
                    tid, base = resid_assign[br]
                    tofs = base + in_chunk
                off = 0
                while off < take:
                    bb = tofs + off
                    lim = take - off if bb == 0 else min(take - off,
                                                         (bb & -bb))
                    plan["pieces"].append((br, vv, tid, tofs + off,
                                           lo + off, lim))
                    off += lim
                used_rows[tid] = max(used_rows[tid], tofs + take)
                lo += take
                remaining -= take
    plan["used_rows"] = used_rows

    # W rows per tile: tile partition row -> (br, original g) or None
    rows = [[None] * 128 for _ in range(n_tiles)]
    for br in range(BR):
        pos = 0
        for (vv, slot_lo, n) in plan["bands"][br]:
            for i in range(n):
                s = slot_lo + i
                g_orig = int(perms[br][pos + i])
                chunk = s // 128
                if chunk == 0:
                    tid, tofs = br, s
                else:
                    tid, base = resid_assign[br]
                    tofs = base + (s % 128)
                rows[tid][tofs] = (br, g_orig)
            pos += n
    plan["tile_rows"] = rows
    return plan


def _schedule():
    """Gather/DMA tile schedule: (br, ko, ki_lo, nki). First tile split for
    pipeline ramp; last ko holds only k 28-29 (row 15 unused)."""
    sched = []
    for ko in range(NKO):
        for br in range(BR):
            if (br, ko) == (0, 0):
                sched += [(br, ko, 0, 2), (br, ko, 2, 2)]
            elif ko == NKO - 1:
                sched.append((br, ko, 0, 2))
            else:
                sched.append((br, ko, 0, KB))
    return sched


def _build_program(plan):
    nc = bacc.Bacc("TRN2", target_bir_lowering=False, debug=False,
                   num_devices=N_CORES)

    S = plan["S"]
    n_tiles = plan["n_tiles"]
    sched = _schedule()

    # idxt total columns
    segs = {}
    col = 0
    for (br, ko, ki_lo, nki) in sched:
        segs[(br, ko, ki_lo)] = col
        col += nki * S[br] // 16
    idx_cols = col

    xs = [nc.dram_tensor(f"x{i}", [NK, 128, C], mybir.dt.float32,
                         kind="ExternalInput") for i in range(BR)]
    idxt_d = nc.dram_tensor("idxt", [128, idx_cols], mybir.dt.int16,
                            kind="ExternalInput")
    wt_d = nc.dram_tensor("wt", [128, n_tiles * OC], mybir.dt.bfloat16,
                          kind="ExternalInput")
    # oc-major output layout; one DMA per row group at the end
    outs_d = [nc.dram_tensor(f"out{g}", [6, 128, B, GWQ[g]], mybir.dt.bfloat16,
                             kind="ExternalOutput") for g in range(NG)]

    f32 = mybir.dt.float32
    bf16 = mybir.dt.bfloat16

    pieces_by_chunk = {}
    for br, vv, tid, tofs, slot_lo, n in plan["pieces"]:
        pieces_by_chunk.setdefault((br, slot_lo // 128), []).append(
            (vv, tid, tofs, slot_lo % 128, n))

    with tile.TileContext(nc) as tc, ExitStack() as ctx:
        cpool = ctx.enter_context(tc.tile_pool(name="const", bufs=1))
        xpool = ctx.enter_context(tc.tile_pool(name="xin", bufs=6))
        ppool = ctx.enter_context(tc.tile_pool(name="picked", bufs=8))
        featp = ctx.enter_context(tc.tile_pool(name="feats", bufs=1))
        opool = ctx.enter_context(tc.tile_pool(name="ostage", bufs=1))
        t2p = ctx.enter_context(tc.tile_pool(name="ps_t2", bufs=3, space="PSUM"))
        cvp = ctx.enter_context(tc.tile_pool(name="ps_cv", bufs=5, space="PSUM"))

        ident = cpool.tile([128, 128], f32)
        masks.make_identity(nc, ident[:])
        idxt = cpool.tile([128, idx_cols], mybir.dt.int16)
        wtb = cpool.tile([128, n_tiles * OC], bf16)

        # feats[g][tid]: [slot, (b, q')] bf16 per conv row group
        feats = [[featp.tile([128, B * GWQ[g]], bf16, name=f"feat{g}_{i}")
                  for i in range(n_tiles)] for g in range(NG)]
        # persistent per-group output staging: [o, (oc, b, q')]
        ots = [opool.tile([128, 6 * B * GWQ[g]], bf16, name=f"ot{g}")
               for g in range(NG)]

        rr = 0  # engine round-robin for copies

        def vcopy(dst, src):
            nonlocal rr
            if rr % 2 == 0:
                nc.vector.tensor_copy(dst, src)
            else:
                nc.scalar.copy(dst, src)
            rr += 1

        # zero only rows above each tile's written extent (pad rows inside
        # the extent have zero W rows)
        for g in range(NG):
            for i in range(n_tiles):
                u = plan["used_rows"][i]
                while u < 128:
                    span = 128 - u if u == 0 else min(128 - u, u & -u)
                    nc.vector.memset(feats[g][i][u:u + span, :], 0.0)
                    u += span

        def conv_unit(g, oc, bg):
            wq = GWQ[g]
            nb = GNB[g]
            pc = cvp.tile([128, nb * wq], f32)
            for t in range(n_tiles):
                lhsT = wtb[:, t * OC + oc * 128:
                           t * OC + (oc + 1) * 128]
                rhs = feats[g][t][:, bg * nb * wq:
                                  (bg + 1) * nb * wq]
                nc.tensor.matmul(pc[:], lhsT, rhs,
                                 start=(t == 0),
                                 stop=(t == n_tiles - 1))
            vcopy(ots[g][:, (oc * B + bg * nb) * wq:
                         (oc * B + (bg + 1) * nb) * wq], pc[:])

        # x DMA issue (SP queue, in schedule order); gather right after
        xts = {}
        for ei, (br, ko, ki_lo, nki) in enumerate(sched):
            xv = xs[br].ap().rearrange("(ko ki) pb c -> ko pb ki c", ki=KB)
            xt = xpool.tile([128, nki * C], f32)
            if ei == 0:
                # index DMA first: the first gather needs it
                nc.sync.dma_start(idxt[:], idxt_d.ap())
            nc.sync.dma_start(xt[:], xv[ko][:, ki_lo:ki_lo + nki, :])
            xts[(br, ko, ki_lo)] = xt
            if ei == 3:
                # W needed from conv g0 (~t=45us) only
                nc.sync.dma_start(wtb[:], wt_d.ap())

        def entry_of(br, ko, ki):
            if (br, ko) in ((0, 0), (BR - 1, NKO - 1)):
                lo = 0 if ki < KB // 2 else KB // 2
            else:
                lo = 0
            return lo

        # conv trigger points: group g's conv units become ready once the
        # entries covering its source rows are processed.  Units are then
        # emitted a few at a time between schedule entries so the in-order
        # PE stream interleaves transposes with conv matmuls.
        # rows per ko: 2ko, 2ko+1.  g0 (qr 0-6) needs rows 0-7 -> ko 3;
        # g1 (qr 7-10) rows 7-11 -> ko 5; g2 (qr 11-12) rows 11-13 -> ko 6;
        # g3 (qr 13) rows 13-14 -> ko 7.
        triggers = {
            (BR - 1, 2, 0): 0,
            (BR - 1, 4, 0): 1,
            (BR - 1, 5, 0): 2,
            (BR - 1, 6, 0): 3,
            (BR - 1, 7, 0): 4,
        }
        pending = []

        def flush_units(k):
            for _ in range(min(k, len(pending))):
                conv_unit(*pending.pop(0))

        # per-tile gather; row-pair transposes + crop copies fire as soon
        # as all four k-blocks of a pair have been gathered (pairs may
        # span split entries); conv units interleave between blocks.
        pk_of = {}   # (br, k) -> (pk tile, ki_rel, sb)
        done_k = {}  # br -> set of gathered k
        for si, (br, ko, ki_lo, nki) in enumerate(sched):
            sb = S[br]
            xt = xts[(br, ko, ki_lo)]
            pk = ppool.tile([128, nki * sb], f32)
            nc.gpsimd.ap_gather(
                pk[:], xt[:],
                idxt[:, segs[(br, ko, ki_lo)]:
                     segs[(br, ko, ki_lo)] + nki * sb // 16],
                channels=128, num_elems=nki * C, d=1, num_idxs=nki * sb)
            ks = done_k.setdefault(br, set())
            for ki_rel in range(nki):
                pk_of[(br, ko * KB + ki_lo + ki_rel)] = (pk, ki_rel)
                ks.add(ko * KB + ki_lo + ki_rel)

            # the ko's row pair (rows 2ko, 2ko+1) once fully gathered;
            # ko7 carries only k 28-29 (row 14; row 15 unused)
            pair_kis = [k for k in range(4 * ko, 4 * ko + 4) if k < NK - 2]
            if all(k in ks for k in pair_kis) and ki_lo + nki == len(pair_kis):
                r0 = 2 * ko
                for c0 in range((sb + 127) // 128):
                    cn = min(128, sb - 128 * c0)
                    plist = pieces_by_chunk.get((br, c0))
                    if not plist:
                        continue
                    pt = t2p.tile([128, 512], f32)
                    for k in pair_kis:
                        pkj, ki_rel = pk_of[(br, k)]
                        jj = k - 4 * ko
                        src = pkj[:, ki_rel * sb + 128 * c0:
                                  ki_rel * sb + 128 * c0 + cn]
                        nc.tensor.transpose(
                            pt[:cn, jj * 128:(jj + 1) * 128], src, ident[:])
                    # psum free layout: (row 2, c 16, b 16); crop copies
                    for vv, tid, tofs, plo, n in plist:
                        dh, dw = vv // 2, vv % 2
                        qr0, qr1 = r0 - dh, r0 + 1 - dh
                        g0 = _qgroup(qr0) if 0 <= qr0 else None
                        g1 = _qgroup(qr1) if qr1 <= 13 else None
                        src4 = pt[plo:plo + n].rearrange(
                            "g (r c b) -> g r c b", r=2, b=16)
                        if g0 is not None and g0 == g1:
                            base = 14 * (qr0 - GLO[g0])
                            dst = feats[g0][tid][tofs:tofs + n].rearrange(
                                "g (b q) -> g q b", b=16)[
                                :, base:base + 28, :].rearrange(
                                "g (r c) b -> g r c b", r=2)
                            vcopy(dst, src4[:, :, dw:dw + 14, :])
                        else:
                            for ri, (qr, gg) in enumerate(((qr0, g0),
                                                           (qr1, g1))):
                                if gg is None:
                                    continue
                                base = 14 * (qr - GLO[gg])
                                dst = feats[gg][tid][tofs:tofs + n].rearrange(
                                    "g (b q) -> g q b", b=16)[
                                    :, base:base + 14, :].rearrange(
                                    "g (r c) b -> g r c b", r=1)
                                vcopy(dst, src4[:, ri:ri + 1, dw:dw + 14, :])
                    flush_units(4 if ko >= 5 else 2)

            g_trig = triggers.get((br, ko, ki_lo))
            if g_trig is not None:
                pending.extend((g_trig, oc, bg) for oc in range(6)
                               for bg in range(B // GNB[g_trig]))
            # throttled mid-stream so conv lumps never delay the in-order
            # PE transposes feeding later groups; greedy only at the end
            flush_units(len(pending) if si == len(sched) - 1
                        else (4 if ko >= 5 else 3))

        # deferred output DMAs: g0 on the SP queue (runs right after the x
        # stream), g1-g3 on other queues so their latencies overlap
        qs = [nc.sync, nc.scalar, nc.scalar, nc.sync, nc.sync]
        for g in range(NG):
            dd = outs_d[g].ap().rearrange("c o b q -> o c b q")
            qs[g].dma_start(dd, ots[g][:].rearrange(
                "o (c b q) -> o c b q", c=6, b=B))

    nc.compile()
    return nc


def _prep_aux(idx, offh, offw, W, plan):
    """Host-side index/layout preprocessing (relayout + address arithmetic)."""
    idx = np.asarray(idx)
    W = np.asarray(W, dtype=np.float32)
    perms = plan["perms"]
    S = plan["S"]
    Smax = max(S)

    # padded sorted index array per branch: [Smax, 256]
    idx_pad = np.zeros((BR, Smax, P), np.int64)
    for br in range(BR):
        pos = 0
        for (vv, slot_lo, n) in plan["bands"][br]:
            idx_pad[br, slot_lo:slot_lo + n] = idx[br][perms[br][pos:pos + n]]
            pos += n

    # gather index tiles per schedule entry: list position t = ki_rel*S + s,
    # value ki_rel*C + idx_pad[br, s, k*8 + j]; stored at partition
    # 16*j + (t%16), free col_offset + t//16.
    sched = _schedule()
    cols = sum(nki * S[br] // 16 for (br, ko, ki_lo, nki) in sched)
    idxt = np.zeros((128, cols), np.int16)
    col = 0
    for (br, ko, ki_lo, nki) in sched:
        sb = S[br]
        L = nki * sb
        vals = np.zeros((8, L), np.int64)
        for ki_rel in range(nki):
            k = ko * KB + ki_lo + ki_rel
            blk = idx_pad[br, :sb, k * 8:(k + 1) * 8]      # [sb, j]
            vals[:, ki_rel * sb:(ki_rel + 1) * sb] = (ki_rel * C + blk).T
        tmp = vals.reshape(8, L // 16, 16).transpose(0, 2, 1)  # [j, r, c]
        idxt[:, col:col + L // 16] = tmp.reshape(128, L // 16)
        col += L // 16

    # W tiles: [g row, tile, o] bf16; zero rows for pad slots
    Wr = W.reshape(OC, BR, 128)                      # [o, br, g]
    n_tiles = plan["n_tiles"]
    wt = np.zeros((128, n_tiles, OC), np.float32)
    for tid in range(n_tiles):
        for row in range(128):
            ent = plan["tile_rows"][tid][row]
            if ent is not None:
                br, g_orig = ent
                wt[row, tid] = Wr[:, br, g_orig]
    wt = np.ascontiguousarray(wt.reshape(128, n_tiles * OC)).astype(
        ml_dtypes.bfloat16)
    return idxt, wt


def _relayout_x(xc):
    """[16, 512, 256] -> [32, 128, 512]: out[k, pl*16+b, c] = x[b, c, 8k+pl]."""
    t = xc.reshape(B, C, NK, 8)                  # [b, c, k, pl]
    t = t.transpose(2, 3, 0, 1)                  # [k, pl, b, c]
    return np.ascontiguousarray(t.reshape(NK, 128, C))


def kernel(x1, x3, x5, W, idx, offh, offw):
    x1 = np.asarray(x1, dtype=np.float32)
    x3 = np.asarray(x3, dtype=np.float32)
    x5 = np.asarray(x5, dtype=np.float32)
    Bfull = x1.shape[0]
    assert Bfull == N_CORES * B

    offh = np.asarray(offh).astype(np.int64)
    offw = np.asarray(offw).astype(np.int64)
    plan = _plan(offh, offw)
    idxt, wt = _prep_aux(idx, offh, offw, W, plan)

    key = (tuple(plan["S"]), plan["n_tiles"],
           tuple(plan["pieces"]))
    if _CACHE.get("key") != key:
        _CACHE["nc"] = _build_program(plan)
        _CACHE["key"] = key
    nc = _CACHE["nc"]

    in_maps = []
    for core in range(N_CORES):
        sl = slice(core * B, (core + 1) * B)
        in_maps.append({
            "x0": _relayout_x(x1[sl].reshape(B, C, P)),
            "x1": _relayout_x(x3[sl].reshape(B, C, P)),
            "x2": _relayout_x(x5[sl].reshape(B, C, P)),
            "idxt": idxt,
            "wt": wt,
        })

    res = bass_utils.run_bass_kernel_spmd(nc, in_maps, list(range(N_CORES)))
    outs = []
    for i in range(N_CORES):
        # out groups: [6, 128, B, wq] bf16, q-rows in order
        o = np.concatenate(
            [np.asarray(res.results[i][f"out{g}"]).astype(
                np.float32).reshape(OC, B, GWQ[g])
             for g in range(NG)], axis=2)          # [OC, B, 196]
        o = o.transpose(1, 0, 2).reshape(B, OC, 14, 14)
        outs.append(o)
    return np.concatenate(outs, axis=0)
